# revision 22
# baseline (speedup 1.0000x reference)
"""Tensor-parallel GQA attention prefill block for 8 Trainium2 NeuronCores.

Problem (hardcoded): x:[2,1024,4096] f32, 32 Q heads / 8 KV heads, head dim
128, RoPE at positions arange(1024), causal mask, KV-cache positions >=1024
masked out (cache starts zeroed), output projection Wo. The computation
reduces exactly to causal GQA attention + o_proj.

Sharding: tensor-parallel over heads. Core c owns Q heads 4c..4c+3 and KV
head c (Wq/Wk/Wv column shards), computes attention for its heads over all
tokens, then AllToAlls exchange attention outputs so each core holds all
4096 features for a 128-token slice per batch; o_proj runs token-sharded
with the full (bf16) Wo; host concatenates the token slices.

Key scheduling facts (from perfetto traces of earlier revisions):
- PE idle gaps > ~3.4us re-throttle the PE clock to 1.2 GHz (HAM). The
  gap sources were: weight-slab DMAs too small/slow (QKV), softmax DVE
  chain too long (attention), and the second batch-1 AllToAll exposed
  (o_proj). Everything below exists to keep PE streaming back-to-back.
- QKV weights ride in pre-transposed [128, 32*256] layout so each DMA
  moves 4 slabs with 2KB/partition lines (vs 512B single-slab loads).
- Softmax: exact-causal score widths, additive -30000 mask on the
  diagonal 128-block (DVE, on PSUM), exp with accum_out for the row sum
  (no separate mask-mul/reduce), diag(1/sum) built on GpSimd. The only
  DVE work left on the PE critical path is tiny.
- o_proj runs in two 2-dmq-column waves; within a wave the fc blocks
  whose features come from the FIRST AllToAll half (fc%4<2) run first,
  giving ~34us of PE cover for the second batch-1 collective.
- All matmuls bf16 with fp32 PSUM accumulation; output written bf16 and
  upcast on host.
"""
import sys

sys.path.insert(0, "/opt/trn_rl_repo")

import numpy as np
import ml_dtypes

import concourse.bass as bass
import concourse.tile as tile
from concourse import mybir
from concourse.bass import ts
from concourse.bass_utils import run_bass_kernel_spmd

BF16 = mybir.dt.bfloat16
F32 = mybir.dt.float32
AF = mybir.ActivationFunctionType
OP = mybir.AluOpType

B, S, D = 2, 1024, 4096
H, KVH, HD = 32, 8, 128
NC = 8
QH = H // NC  # 4 q heads per core
THETA = 1000000.0
SC = 1.0 / float(np.sqrt(HD))

RG = [list(range(NC))]


def _build(split_for_walrus=True):
    nc = bass.Bass("TRN2", num_devices=NC)

    xT = nc.declare_dram_parameter("xT", [B, D, S], BF16, isOutput=False)
    # pre-transposed weight slabs: [128 rows-in-slab, 32 slabs * 256 cols]
    wA = nc.declare_dram_parameter("wA", [128, 8192], BF16, isOutput=False)
    wB = nc.declare_dram_parameter("wB", [128, 8192], BF16, isOutput=False)
    wC = nc.declare_dram_parameter("wC", [128, 8192], BF16, isOutput=False)
    wo = nc.declare_dram_parameter("wo", [D, D], BF16, isOutput=False)
    bias_row = nc.declare_dram_parameter("bias_row", [1, 768], BF16, isOutput=False)
    ones_row = nc.declare_dram_parameter("ones_row", [1, S], BF16, isOutput=False)
    cosT = nc.declare_dram_parameter("cosT", [128, S], BF16, isOutput=False)
    sinT = nc.declare_dram_parameter("sinT", [128, S], BF16, isOutput=False)
    maskadd = nc.declare_dram_parameter("maskadd", [128, 128], BF16, isOutput=False)
    ident = nc.declare_dram_parameter("ident", [128, 128], BF16, isOutput=False)
    out = nc.declare_dram_parameter("out", [B, 128, D], BF16, isOutput=True)

    from contextlib import ExitStack

    with ExitStack() as es:
        tc = es.enter_context(tile.TileContext(nc))
        cpool = es.enter_context(tc.tile_pool(name="consts", bufs=1))
        xcpool = es.enter_context(tc.tile_pool(name="xc", bufs=40))
        wpool = es.enter_context(tc.tile_pool(name="wslab", bufs=6))
        ropepool = es.enter_context(tc.tile_pool(name="rope", bufs=2))
        qrotpool = es.enter_context(tc.tile_pool(name="qrot", bufs=6))
        vtpool = es.enter_context(tc.tile_pool(name="vt", bufs=2))
        ppool = es.enter_context(tc.tile_pool(name="attn", bufs=16))
        ptsbpool = es.enter_context(tc.tile_pool(name="ptsb", bufs=9))
        sumpool = es.enter_context(tc.tile_pool(name="sums", bufs=16))
        atpool = es.enter_context(tc.tile_pool(name="at", bufs=4))
        gpool = es.enter_context(tc.tile_pool(name="g", bufs=2))
        dgpool = es.enter_context(tc.tile_pool(name="diag", bufs=16))
        ypool = es.enter_context(tc.tile_pool(name="ysb", bufs=4))
        psA = es.enter_context(tc.tile_pool(name="psA", bufs=3, space="PSUM"))
        psB = es.enter_context(tc.tile_pool(name="psB", bufs=2, space="PSUM"))
        dpool = es.enter_context(tc.tile_pool(name="dram", bufs=4, space="DRAM"))
        if True:
            # constants ride the vector queue so the first weight chunk and
            # x tile are at the head of the scalar/sync queues.
            cos_sb = cpool.tile([128, S], BF16, tag="cos", name="cos")
            sin_sb = cpool.tile([128, S], BF16, tag="sin", name="sin")
            madd_sb = cpool.tile([128, 128], BF16, tag="maskadd", name="maskadd")
            id_sb = cpool.tile([128, 128], BF16, tag="ident", name="ident")
            br_sb = cpool.tile([1, 768], BF16, tag="biasr", name="biasr")
            on_sb = cpool.tile([1, S], BF16, tag="ones", name="ones")
            nc.gpsimd.dma_start(cos_sb[:], cosT[:])
            nc.gpsimd.dma_start(sin_sb[:], sinT[:])
            nc.gpsimd.dma_start(madd_sb[:], maskadd[:])
            nc.gpsimd.dma_start(id_sb[:], ident[:])
            nc.gpsimd.dma_start(br_sb[:], bias_row[:])
            nc.gpsimd.dma_start(on_sb[:], ones_row[:])

            G = [None, None]

            for b in range(B):
                # ---- QKV projection + RoPE for batch b ----
                xc = []
                for k in range(32):
                    t = xcpool.tile([128, S], BF16, tag="xc", name="xc")
                    if b == 0 and k < 2:
                        nc.sync.dma_start(t[:, 0:512], xT[b, ts(k, 128), 0:512])
                        nc.sync.dma_start(t[:, 512:1024], xT[b, ts(k, 128), 512:1024])
                    else:
                        nc.sync.dma_start(t[:], xT[b, ts(k, 128), :])
                    xc.append(t)

                rope_out = {}  # mg -> rotated tile
                v_sb = vtpool.tile([128, S], BF16, tag="v", name="v")
                for grp, wparam in ((0, wA), (1, wB), (2, wC)):
                    pst = [
                        psA.tile([128, S], F32, tag="A", name="pst") for _ in range(2)
                    ]
                    for kc4 in range(8):
                        wch = wpool.tile([128, 1024], BF16, tag="wslab", name="wch")
                        nc.scalar.dma_start(wch[:], wparam[:, ts(kc4, 1024)])
                        for k4 in range(4):
                            k = kc4 * 4 + k4
                            for m in range(2):
                                for n in range(2):
                                    nc.tensor.matmul(
                                        pst[m][:, ts(n, 512)],
                                        wch[:, ts(k4, 256)][:, ts(m, 128)],
                                        xc[k][:, ts(n, 512)],
                                        start=(k == 0),
                                        stop=False,
                                    )
                    for m in range(2):
                        mg = grp * 2 + m  # 0=Q0 1=K 2=V 3=Q1 4=Q2 5=Q3
                        for n in range(2):
                            nc.tensor.matmul(
                                pst[m][:, ts(n, 512)],
                                br_sb[0:1, ts(mg, 128)],
                                on_sb[0:1, ts(n, 512)],
                                start=False, stop=(n == 1),
                            )
                        if mg != 2:
                            q32 = ropepool.tile([128, S], BF16, tag="q32", name="q32")
                            nc.vector.tensor_copy(q32[:], pst[m][:])
                            sh = ropepool.tile([128, S], BF16, tag="sh", name="sh")
                            nc.sync.dma_start(sh[0:64, :], q32[64:128, :])
                            nc.sync.dma_start(sh[64:128, :], q32[0:64, :])
                            eng = nc.vector
                            eng.tensor_mul(q32[:], q32[:], cos_sb[:])
                            eng.tensor_mul(sh[:], sh[:], sin_sb[:])
                            rot = qrotpool.tile([128, S], BF16, tag="qrot", name="qrot")
                            eng.tensor_add(rot[:], q32[:], sh[:])
                            rope_out[mg] = rot
                        else:
                            vt = vtpool.tile([128, S], BF16, tag="vt", name="vt")
                            nc.vector.tensor_copy(vt[:], pst[m][:])
                            for j in range(8):
                                vp = psB.tile([128, 128], F32, tag="B", name="vp")
                                nc.tensor.matmul(
                                    vp[:], vt[:, ts(j, 128)], id_sb[:],
                                    start=True, stop=True,
                                )
                                if j % 2 == 0:
                                    nc.vector.tensor_copy(
                                        v_sb[:, ts(j, 128)], vp[:]
                                    )
                                else:
                                    nc.scalar.copy(v_sb[:, ts(j, 128)], vp[:])

                K_t = rope_out[1]
                q_heads = [rope_out[0], rope_out[3], rope_out[4], rope_out[5]]

                # ---- attention: software-pipelined over (head, group) units.
                # Exact-causal score widths; additive diagonal mask in PSUM;
                # exp+accum_out row sums; diag(1/sum) on GpSimd. PE stream per
                # step: PT(prev) -> scores(cur) -> OT(prev).
                at = [
                    atpool.tile([128, S], BF16, tag="at", name="at")
                    for _ in range(QH)
                ]

                def emit_scores_softmax(h, g):
                    Q_t = q_heads[h]
                    plist = []
                    for j in range(4):
                        qi = 4 * g + j
                        W = (qi + 1) * 128  # causal width for this q block
                        sp = psA.tile([128, W], F32, tag="A", name="sp")
                        nmm = (W + 511) // 512
                        for c in range(nmm):
                            n0, n1 = c * 512, min(W, (c + 1) * 512)
                            nc.tensor.matmul(
                                sp[:, n0:n1],
                                Q_t[:, ts(qi, 128)],
                                K_t[:, n0:n1],
                                start=True, stop=(n1 < W),
                            )
                        # strict-upper-triangle -30000 on the diagonal block,
                        # accumulated by PE (I.T @ U) so no DVE hop sits
                        # between the scores matmul and the exp
                        nc.tensor.matmul(
                            sp[:, qi * 128 : W], id_sb[:], madd_sb[:],
                            start=False, stop=True,
                        )
                        P = ppool.tile([128, W], BF16, tag="psb", name="psb")
                        sums = sumpool.tile([128, 1], F32, tag="sums", name="sums")
                        nc.scalar.activation(
                            P[:], sp[:, 0:W], AF.Exp, scale=SC, accum_out=sums[:]
                        )
                        recip = sumpool.tile([128, 1], F32, tag="recip", name="recip")
                        nc.vector.reciprocal(recip[:], sums[:])
                        Dt = dgpool.tile([128, 128], BF16, tag="diag", name="diag")
                        nc.vector.tensor_scalar_mul(Dt[:], id_sb[:], recip[:])
                        plist.append((P, Dt))
                    return plist

                def emit_pt(g, plist):
                    pts = []
                    for kc in range(4 * g + 4):
                        jst = max(0, kc - 4 * g)
                        ptp = psB.tile([128, 512], F32, tag="B", name="ptp")
                        for j in range(jst, 4):
                            nc.tensor.matmul(
                                ptp[:, ts(j, 128)],
                                plist[j][0][:, ts(kc, 128)],
                                plist[j][1][:],
                                start=True, stop=True,
                            )
                        pt = ptsbpool.tile([128, 512], BF16, tag="ptsb", name="ptsb")
                        if kc % 2 == 0:
                            nc.vector.tensor_copy(
                                pt[:, jst * 128 : 512], ptp[:, jst * 128 : 512]
                            )
                        else:
                            nc.scalar.copy(
                                pt[:, jst * 128 : 512], ptp[:, jst * 128 : 512]
                            )
                        pts.append((pt, jst))
                    return pts

                def emit_ot(h, g, pts):
                    ot = psA.tile([128, 512], F32, tag="A", name="ot")
                    nkc = 4 * g + 4
                    for kc in range(nkc):
                        pt, jst = pts[kc]
                        nc.tensor.matmul(
                            ot[:, jst * 128 : 512],
                            v_sb[:, ts(kc, 128)],
                            pt[:, jst * 128 : 512],
                            start=(kc == 0), stop=(kc == nkc - 1),
                        )
                    nc.vector.tensor_copy(at[h][:, ts(g, 512)], ot[:])

                order = [(h, g) for h in range(QH) for g in range(2)]
                plists = {
                    0: emit_scores_softmax(*order[0]),
                    1: emit_scores_softmax(*order[1]),
                    2: emit_scores_softmax(*order[2]),
                }
                for i in range(len(order)):
                    if i + 3 < len(order):
                        plists[i + 3] = emit_scores_softmax(*order[i + 3])
                    pts = emit_pt(order[i][1], plists.pop(i))
                    emit_ot(order[i][0], order[i][1], pts)

                # ---- AllToAll, split in two half-head collectives: the
                # first half's exchange overlaps whatever compute follows,
                # and o_proj can start on its features before the second
                # half lands ----
                gt = gpool.tile([128, 4096], BF16, tag="g", name="g")
                gtv = gt[:].rearrange("p (s fl t) -> p s fl t", s=NC, fl=QH)
                for half in range(2):
                    a2a_in = dpool.tile(
                        [NC, 256, 128], BF16, tag="a2ain", name="a2ain"
                    )
                    for hl in range(2):
                        nc.gpsimd.dma_start(
                            a2a_in[:].rearrange("d (hh p) t -> hh p d t", hh=2)[hl],
                            at[2 * half + hl][:].rearrange("p (d t) -> p d t", d=NC),
                        )
                    a2a_out = dpool.tile(
                        [NC, 256, 128], BF16, tag="a2aout", name="a2aout"
                    )
                    nc.gpsimd.collective_compute(
                        "AllToAll",
                        OP.bypass,
                        ins=[a2a_in[:].opt()],
                        outs=[a2a_out[:].opt()],
                        replica_groups=RG,
                    )
                    for fl in range(2):
                        nc.gpsimd.dma_start(
                            gtv[:, :, 2 * half + fl, :],
                            a2a_out[:].rearrange(
                                "s (fl p) t -> fl p s t", fl=2
                            )[fl],
                        )
                G[b] = gt

            # ---- token-sharded o_proj with full Wo ----
            # Two waves of 2 dmq columns each; 4 accumulators per wave
            # (3 on psA as [128,1024], 1 split across the two psB banks).
            # fc%4<2 blocks (first-AllToAll features) run first so the
            # second batch-1 collective gets ~16 fc units of PE cover; b0
            # runs before b1 within each fc.
            half_order = [fc for fc in range(32) if fc % 4 < 2] + [
                fc for fc in range(32) if fc % 4 >= 2
            ]
            for w in range(2):
                ypA = [
                    psA.tile([128, 1024], F32, tag="A", name="yp")
                    for _ in range(3)
                ]  # (b0,d0) (b0,d1) (b1,d0)
                ypB = [
                    psB.tile([128, 512], F32, tag="B", name="ypb")
                    for _ in range(2)
                ]  # (b1,d1) n-halves
                OFF = 8  # b0 runs OFF fc-blocks ahead of b1 (collective cover)
                wots = {}

                def emit_oproj_mms(i, fc, b):
                    w0, w1 = wots[i]
                    st, fin = (i == 0), (i == 31)
                    for d in range(2):
                        wt = w0 if d == 0 else w1
                        for n in range(2):
                            ai = b * 2 + d
                            if ai < 3:
                                dst = ypA[ai][:, ts(n, 512)]
                            else:
                                dst = ypB[n][:]
                            nc.tensor.matmul(
                                dst,
                                G[b][:, ts(fc, 128)],
                                wt[:, ts(n, 512)],
                                start=st, stop=fin,
                            )

                def drain(b, d):
                    ys = ypool.tile([128, 1024], BF16, tag="ysb", name="ys")
                    ai = b * 2 + d
                    if ai < 3:
                        if d == 0:
                            nc.scalar.copy(ys[:], ypA[ai][:])
                        else:
                            nc.vector.tensor_copy(ys[:], ypA[ai][:])
                    else:
                        nc.scalar.copy(ys[:, 0:512], ypB[0][:])
                        nc.vector.tensor_copy(ys[:, 512:1024], ypB[1][:])
                    qeng = nc.scalar
                    qeng.dma_start(
                        out[b, :, (2 * w + d) * 1024 : (2 * w + d + 1) * 1024],
                        ys[:],
                    )

                for i in range(32 + OFF):
                    if i < 32:
                        fc = half_order[i]
                        # wo slabs ride the (dead-by-now) xc ring: 40 bufs of
                        # [128,1024] = a 16-slab-deep prefetch window that
                        # fills during the DMA-idle attention phase
                        w0 = xcpool.tile([128, 1024], BF16, tag="xc", name="xc")
                        w1 = xcpool.tile([128, 1024], BF16, tag="xc", name="xc")
                        nc.sync.dma_start(
                            w0[:], wo[ts(fc, 128), w * 2048 : w * 2048 + 1024]
                        )
                        nc.scalar.dma_start(
                            w1[:], wo[ts(fc, 128), w * 2048 + 1024 : w * 2048 + 2048]
                        )
                        wots[i] = (w0, w1)
                        emit_oproj_mms(i, fc, 0)
                        if i == 31:
                            # b0 accumulation is complete here; draining now
                            # frees 2/3 psA buffers before the b1 tail ends,
                            # so wave1's accumulators open without a stall
                            drain(0, 0)
                            drain(0, 1)
                    if i >= OFF:
                        emit_oproj_mms(i - OFF, half_order[i - OFF], 1)
                        del wots[i - OFF]
                drain(1, 0)
                drain(1, 1)

    if split_for_walrus:
        _split_waits(nc, cap=1)
    return nc


def _split_waits(nc, cap=1):
    """This walrus build accepts at most one sync wait per instruction; hoist
    the excess onto same-engine NoOps inserted immediately before."""
    for fn in nc.m.functions:
        for bb in fn.blocks:
            new_insts = []
            for inst in bb.instructions:
                si = inst.sync_info
                if si is not None and si.on_wait and len(si.on_wait) > cap:
                    waits = list(si.on_wait)
                    head, rest = waits[: len(waits) - cap], waits[len(waits) - cap:]
                    for i in range(0, len(head), cap):
                        nop = mybir.InstNoOp(
                            name=f"{inst.name}-wsplit{i}", ins=[], outs=[]
                        )
                        nop.engine = inst.engine
                        nop.sync_info = mybir.SyncInfo(
                            on_wait=head[i : i + cap], on_update=[]
                        )
                        new_insts.append(nop)
                    inst.sync_info = mybir.SyncInfo(
                        on_wait=rest, on_update=list(si.on_update)
                    )
                new_insts.append(inst)
            bb.instructions = new_insts
    return nc


_NC_CACHE = None


def _get_nc():
    global _NC_CACHE
    if _NC_CACHE is None:
        _NC_CACHE = _build()
    return _NC_CACHE


def _prep_inputs(x, storage_idx, Wq, bq, Wk, bk, Wv, bv, Wo):
    bf = ml_dtypes.bfloat16
    xT = np.ascontiguousarray(
        np.asarray(x, np.float32).transpose(0, 2, 1)
    ).astype(bf)  # [B, D, S]
    wo_bf = np.ascontiguousarray(np.asarray(Wo, np.float32)).astype(bf)

    pos = np.asarray(storage_idx, np.int64).astype(np.float32)  # [S]
    inv = (1.0 / (THETA ** (np.arange(0, HD, 2, dtype=np.float32) / HD))).astype(
        np.float32
    )
    fr = pos[:, None] * inv[None, :]  # [S, 64]
    emb = np.concatenate([fr, fr], axis=1)  # [S, HD]
    cosT = np.ascontiguousarray(np.cos(emb).T.astype(np.float32)).astype(bf)  # [HD, S]
    sinT32 = np.ascontiguousarray(np.sin(emb).T).astype(np.float32)
    sinT32[0:64] *= -1.0
    sinT = sinT32.astype(bf)  # fold rotate_half sign

    # additive causal mask for the diagonal 128-block
    r = np.arange(128)[:, None]
    c = np.arange(128)[None, :]
    maskadd = np.where(c <= r, 0.0, -30000.0).astype(np.float32).astype(bf)
    identity = np.eye(128, dtype=np.float32).astype(bf)

    def slabT(w):
        # [4096, 256] -> [128, 32*256] pre-transposed slab layout
        w = np.asarray(w, np.float32).reshape(32, 128, 256).transpose(1, 0, 2)
        return np.ascontiguousarray(w.reshape(128, 8192)).astype(bf)

    in_maps = []
    for core in range(NC):
        q0 = core * 512
        kv = slice(core * 128, (core + 1) * 128)
        wA = slabT(np.concatenate([Wq[:, q0 : q0 + 128], Wk[:, kv]], axis=1))
        wB = slabT(np.concatenate([Wv[:, kv], Wq[:, q0 + 128 : q0 + 256]], axis=1))
        wC = slabT(Wq[:, q0 + 256 : q0 + 512])
        bias_row = np.concatenate(
            [
                np.asarray(bq[q0 : q0 + 128], np.float32),
                np.asarray(bk[core * 128 : (core + 1) * 128], np.float32),
                np.asarray(bv[core * 128 : (core + 1) * 128], np.float32),
                np.asarray(bq[q0 + 128 : q0 + 256], np.float32),
                np.asarray(bq[q0 + 256 : q0 + 384], np.float32),
                np.asarray(bq[q0 + 384 : q0 + 512], np.float32),
            ]
        )[None, :].astype(bf)  # [1, 768]
        ones_row = np.ones((1, 1024), np.float32).astype(bf)
        in_maps.append(
            {
                "xT": xT,
                "wA": wA,
                "wB": wB,
                "wC": wC,
                "wo": wo_bf,
                "bias_row": np.ascontiguousarray(bias_row),
                "ones_row": ones_row,
                "cosT": cosT,
                "sinT": sinT,
                "maskadd": maskadd,
                "ident": identity,
            }
        )
    return in_maps


_LAST_RESULTS = None


def kernel(x, storage_idx, cache, mask, Wq, bq, Wk, bk, Wv, bv, Wo):
    """Full-input, full-output entry point. cache/mask are consumed implicitly:
    cache is zeros and positions >= S are causally masked, so the computation
    reduces to causal attention over the S prefill tokens."""
    global _LAST_RESULTS
    in_maps = _prep_inputs(x, storage_idx, Wq, bq, Wk, bk, Wv, bv, Wo)
    nc = _get_nc()
    res = run_bass_kernel_spmd(nc, in_maps, core_ids=list(range(NC)))
    _LAST_RESULTS = res
    full = np.empty((B, S, D), np.float32)
    for c in range(NC):
        o = res.results[c]["out"]  # [B, 128, D] bf16
        for b in range(B):
            full[b, 128 * c : 128 * (c + 1), :] = np.asarray(o[b], np.float32)
    return full
